# revision 44
# baseline (speedup 1.0000x reference)
"""Trainium2 Bass kernel for nn_Attn_47072841564500 (sparse_attention).

Reference computation:
    proj   = einsum('sbn,mn->sbm', encoder_outputs, W) + b     # [S, B, N]
    scores = einsum('bn,sbn->bs', hidden[0], proj)             # [B, S]
    attn   = softmax(scores, axis=1)[:, None, :]               # [B, 1, S]

Key algebraic reduction: scores[b,s] = sum_n enc[s,b,n] * u[b,n] + hidden[b]@bias
with u = hidden[0] @ W.  The bias term is constant per softmax row, and softmax
is shift-invariant, so it drops entirely.  This removes the [S,B,N] projection
(274 GFLOP -> 0.4 GFLOP) and makes the kernel purely HBM-bandwidth-bound on a
single streaming pass over encoder_outputs.

Distribution: batch (B=64) data-parallel over 8 cores, 8 batch rows per core.
encoder_outputs/hidden are split on B, W is replicated; softmax is per-row so
no cross-device communication is needed.

Per-core dataflow (engine-balanced pipeline; sustains the full ~410 GB/s
per-core HBM rate when the device is quiet):
  - W is loaded first, split across BOTH HWDGE rings (sync+scalar) so nothing
    competes with it, one tile per 128-row chunk so u-matmuls start as chunks
    land; PE is pre-warmed with dummy matmuls so the real chain runs at the
    high clock (cold-PE matmuls measured 2x slower).
  - enc streams in [128, sblk, 1024] tiles (s on partitions, n on free),
    alternating the two HWDGE rings (a single ring head-of-line blocks on
    tile-buffer waits; gpsimd's in-order queue must stay off the enc stream).
    Block sizes are tapered: small first blocks (compute starts earlier) and
    small last blocks (shorter serial tail).
  - Compute is split across two engines so neither carries the whole
    2 ops/elem load: per 4-block tile, 2-3 blocks go DVE-multiply +
    ScalarE activation(Identity, accum_out) reduce, the rest run as fused
    DVE scalar_tensor_tensor with accumulator.  Either engine alone caps at
    ~364-381 GB/s equivalent; the split keeps both under the DMA rate.
  - Softmax stays entirely in SBUF in [128, ST] layout (no DRAM bounce):
    row max/sum use DVE free-dim reduces + gpsimd partition_all_reduce
    (~0.4us), exp on ScalarE (exp/identity/copy share one act table -- no
    reloads), final scale on ScalarE, scatter-out via SWDGE.  Per-batch
    scores tiles rotate through a pool so batch bi+1's accumulation never
    WAR-waits on batch bi's softmax reads.  Only the last batch's chain is
    exposed in the tail (~4us vs ~24us for a DRAM bounce+reload softmax).
"""

import os
import sys

import numpy as np

for _p in ("/root/.axon_site/_ro/trn_rl_repo", "/opt/trn_rl_repo"):
    if os.path.isdir(_p) and _p not in sys.path:
        sys.path.append(_p)

from contextlib import ExitStack

import concourse.bacc as bacc
import concourse.tile as tile
from concourse import mybir
from concourse.bass_isa import ReduceOp

F32 = mybir.dt.float32

S, B, N = 2048, 64, 1024
NCORES = 8
BPC = B // NCORES  # batches per core

# Per-batch block schedule over the ST=16 s-columns.  Small first blocks for
# b=0 (first compute waits on u anyway), small last blocks for the final
# batch (shortens the serial tail after the last enc byte lands).
MAXBLK = 4


def _block_schedule(bi, bpc, st):
    if bi == 0:
        sched = [1, 1, 2] + [MAXBLK] * ((st - 4) // MAXBLK)
    elif bi == bpc - 1:
        sched = [MAXBLK] * ((st - 4) // MAXBLK) + [2, 1, 1]
    else:
        sched = [MAXBLK] * (st // MAXBLK)
    assert sum(sched) == st
    return sched


def build(s=S, bpc=BPC, n=N, mode="mixed", enc_bufs=6, prod_bufs=2):
    """Build the per-core Bass program (SPMD; identical on all cores).

    Per [128,1024] block the measured costs are: DVE fused STT (mult+accum)
    ~1375ns, DVE bare TT mult ~1122ns, ScalarE activation-reduce ~1426ns.
    mode="mixed": half of each tile's blocks go DVE-STT, half go
    DVE-TT + ScalarE-reduce, so both engines stay under the DMA rate.
    mode="split": all reduces on ScalarE.  mode="stt": everything on DVE.
    """
    P = 128
    assert s % P == 0 and n % P == 0 and n % 512 == 0
    ST = s // P        # number of s-tiles (free-dim column per s-tile)
    KC = n // P        # contraction chunks for u = h @ W
    FB = n // 512      # psum free-dim blocks (fp32 moving max = 512)

    nc = bacc.Bacc("TRN2", target_bir_lowering=False, debug=False)
    enc = nc.declare_dram_parameter("enc", [s, bpc, n], F32, isOutput=False)
    hT = nc.declare_dram_parameter("hT", [n, bpc], F32, isOutput=False)
    w = nc.declare_dram_parameter("w", [n, n], F32, isOutput=False)
    out = nc.declare_dram_parameter("out", [bpc, s], F32, isOutput=True)

    with ExitStack() as ctx:
        tc = ctx.enter_context(tile.TileContext(nc))
        singles = ctx.enter_context(tc.tile_pool(name="singles", bufs=1))
        psum_pool = ctx.enter_context(tc.tile_pool(name="psum", bufs=1, space="PSUM"))
        psum_bc = ctx.enter_context(tc.tile_pool(name="psumbc", bufs=2, space="PSUM"))

        # h_sb[p, c, b] = hidden[b, c*128 + p] -- FIRST (the b0 matmul chain
        # needs it; keep its completion out of the W critical path)
        h_sb = singles.tile([P, KC, bpc], F32)
        nc.gpsimd.dma_start(out=h_sb, in_=hT.rearrange("(c p) b -> p c b", p=P))
        # --- W next, split across BOTH HWDGE rings: nothing else is queued
        # yet, so W gets the full DMA bus (it gates the whole u chain).
        # One tile PER chunk: a single multi-DMA tile would make the first
        # matmul wait for ALL of W (whole-tile dependency); separate tiles
        # let matmul(c) start the moment chunk c lands.
        w_r = w.rearrange("(c p) n -> p c n", p=P)
        w_sb = []
        for c in range(KC):
            wt = singles.tile([P, n], F32, tag=f"w{c}", name=f"w_sb{c}")
            eng = nc.sync if c % 2 == 0 else nc.scalar
            eng.dma_start(out=wt, in_=w_r[:, c, :])
            w_sb.append(wt)
        ones_sb = singles.tile([1, P], F32)
        nc.vector.memset(ones_sb, 1.0)

        # --- b=0's u, broadcast to all partitions directly on TensorE:
        # stationary = h[0, m-chunk] replicated across all 128 M columns,
        # moving = W chunk.  c-outer so accumulation completes right after
        # the last W chunk lands -- this is the first-compute critical path.
        # (The scheduler coarsens the first matmul's wait to cover ALL the W
        # chunks regardless of chunking, so first compute lands ~32us in;
        # attempts to split the accumulation into half-W groups don't help.)
        psum_ubc0 = psum_pool.tile([P, 1, n], F32, tag="ubc0")
        # PE p-state warmup: the first real matmuls otherwise run at the low
        # clock (measured 1219ns vs 592ns warm).  Burn dep-free dummy
        # matmuls during the NEFF preamble / W load so the clock is ramped
        # by the time W chunk 0 lands.  start=True on the first real matmul
        # reinitializes the accumulator, so scribbling the tile is safe.
        for _ in range(20):
            nc.tensor.matmul(
                psum_ubc0[:, 0, 0:P],
                lhsT=ones_sb,
                rhs=ones_sb,
                start=True,
                stop=True,
            )
        for c in range(KC):
            for fb in range(FB):
                fsl = slice(fb * 512, (fb + 1) * 512)
                nc.tensor.matmul(
                    psum_ubc0[:, 0, fsl],
                    lhsT=h_sb[:, c, 0:1].to_broadcast([P, P]),
                    rhs=w_sb[c][:, fsl],
                    start=(c == 0),
                    stop=(c == KC - 1),
                )

        # --- u[b, n'] for all b (M=8; same moving-column cost) ---
        psum_u = psum_pool.tile([bpc, n], F32, tag="u")
        for c in range(KC):
            for fb in range(FB):
                fsl = slice(fb * 512, (fb + 1) * 512)
                nc.tensor.matmul(
                    psum_u[:, fsl],
                    lhsT=h_sb[:, c, :],
                    rhs=w_sb[c][:, fsl],
                    start=(c == 0),
                    stop=(c == KC - 1),
                )
        u_rows = singles.tile([bpc, n], F32)
        nc.scalar.copy(u_rows, psum_u)
        # PE moving data must start at partition 0/32/64, so rows 1..7 are
        # relocated to partition 0 with ONE small SWDGE gather for the
        # per-b ones-outer-product broadcasts.
        u_r0 = singles.tile([1, bpc - 1, n], F32)
        nc.gpsimd.dma_start(out=u_r0[0:1, :, :], in_=u_rows[1:, :])

        encp = ctx.enter_context(tc.tile_pool(name="encp", bufs=enc_bufs))
        prodp = ctx.enter_context(tc.tile_pool(name="prodp", bufs=prod_bufs))
        dumpp = ctx.enter_context(tc.tile_pool(name="dump", bufs=1))
        dump = dumpp.tile([P, n], F32)
        scrp = ctx.enter_context(tc.tile_pool(name="scr", bufs=2))
        smp = ctx.enter_context(tc.tile_pool(name="smp", bufs=2))

        # s index mapping: s = p*ST + st (partition-major) so per-b output
        # rows [128, ST] scatter contiguously into out[b, :].
        enc_r = enc.rearrange("(p st) b n -> p st b n", p=P)
        out_r = out.rearrange("b (p st) -> p b st", p=P)

        # Per-batch scores tiles from a rotating pool: a single shared tile
        # would make batch bi+1's first accumulator write WAR-wait on batch
        # bi's softmax reads, stalling every batch transition.
        scorep = ctx.enter_context(tc.tile_pool(name="scorep", bufs=3))

        dma_i = 0
        tile_i = 0
        for bi in range(bpc):
            scores = scorep.tile([P, ST], F32, tag="scores")
            if bi == 0:
                psum_ubc = psum_ubc0
            else:
                # u_bc[p, n'] = u[bi, n'] broadcast to all partitions via a
                # K=1 outer-product matmul: ones[1,128].T @ u_r0[0:1, bi-1, fsl]
                psum_ubc = psum_bc.tile([P, 1, n], F32, tag="ubc")
                for fb in range(FB):
                    fsl = slice(fb * 512, (fb + 1) * 512)
                    nc.tensor.matmul(
                        psum_ubc[:, 0, fsl],
                        lhsT=ones_sb,
                        rhs=u_r0[0:1, bi - 1, fsl],
                        start=True,
                        stop=True,
                    )

            st0 = 0
            for sblk in _block_schedule(bi, bpc, ST):
                et = encp.tile([P, MAXBLK, n], F32)
                # 1:1 across the two HWDGE rings (a SINGLE ring head-of-line
                # blocks on tile-buffer waits and loses ~15% bandwidth; the
                # second ring provides the needed 2-deep parallelism).
                # gpsimd stays OFF the enc stream: its in-order queue would
                # head-of-line block enc DMAs behind softmax all-reduces.
                eng = nc.sync if dma_i % 2 == 0 else nc.scalar
                dma_i += 1
                eng.dma_start(
                    out=et[:, :sblk, :],
                    in_=enc_r[:, st0 : st0 + sblk, bi, :],
                )
                # First nsplit blocks: DVE bare multiply + ScalarE reduce.
                # Rest: DVE fused multiply+reduce.  Balances the two engines
                # (per-block: DVE fused ~1.3us, DVE bare mult ~1.1us,
                # ScalarE reduce ~1.4us; DMA delivers a block per ~1.28us).
                # Keep ScalarE under ~75% so its queue-depth-0 sequencer can
                # absorb the enc-DMA-issue buffer waits without stalling.
                if mode == "split":
                    nsplit = sblk
                elif mode == "stt":
                    nsplit = 0
                elif sblk == MAXBLK:
                    # 1-in-3 tiles push all 4 reduces to ScalarE: raises
                    # DVE's catch-up rate over the stream (it starts ~24us
                    # after DMA) while keeping ScalarE, including its
                    # ~667ns/issue DMA dispatches, under ~96% of the DMA
                    # period.  ([4,3] alternating would put ScalarE over.)
                    nsplit = 4 if tile_i % 3 == 0 else 3
                    tile_i += 1
                else:
                    nsplit = sblk // 2
                if nsplit:
                    prod = prodp.tile([P, MAXBLK, n], F32, tag="prod")
                    nc.vector.tensor_tensor(
                        out=prod[:, :nsplit, :],
                        in0=et[:, :nsplit, :],
                        in1=psum_ubc[:, 0:1, :].to_broadcast([P, nsplit, n]),
                        op=mybir.AluOpType.mult,
                    )
                    for j in range(nsplit):
                        nc.scalar.activation(
                            out=dump,
                            in_=prod[:, j, :],
                            func=mybir.ActivationFunctionType.Identity,
                            accum_out=scores[:, st0 + j : st0 + j + 1],
                        )
                for j in range(nsplit, sblk):
                    dumpt = scrp.tile([P, n], F32, tag="dump")
                    nc.vector.scalar_tensor_tensor(
                        out=dumpt,
                        in0=et[:, j, :],
                        scalar=0.0,
                        in1=psum_ubc[:, 0, :],
                        op0=mybir.AluOpType.add,
                        op1=mybir.AluOpType.mult,
                        accum_out=scores[:, st0 + j : st0 + j + 1],
                    )
                st0 += sblk

            # --- softmax for batch bi, entirely in SBUF in [128, ST] layout.
            # Row stats need a cross-partition step: partition_all_reduce.
            pm = smp.tile([P, 1], F32, tag="pm")
            nc.vector.reduce_max(out=pm, in_=scores, axis=mybir.AxisListType.X)
            gm = smp.tile([P, 1], F32, tag="gm")
            nc.gpsimd.partition_all_reduce(gm, pm, P, ReduceOp.max)
            ngm = smp.tile([P, 1], F32, tag="ngm")
            nc.vector.tensor_scalar_mul(ngm, gm, -1.0)
            expsc = smp.tile([P, ST], F32, tag="expsc")
            es = smp.tile([P, 1], F32, tag="es")
            nc.scalar.activation(
                out=expsc,
                in_=scores,
                func=mybir.ActivationFunctionType.Exp,
                bias=ngm,
                scale=1.0,
                accum_out=es,
            )
            ssum = smp.tile([P, 1], F32, tag="ssum")
            nc.gpsimd.partition_all_reduce(ssum, es, P, ReduceOp.add)
            inv = smp.tile([P, 1], F32, tag="inv")
            nc.vector.reciprocal(inv, ssum)
            outsc = smp.tile([P, ST], F32, tag="outsc")
            # scale on ScalarE (activation Copy w/ per-partition scale) --
            # keeps the busier DVE out of the per-batch softmax chain
            nc.scalar.mul(outsc, expsc, inv)
            # Mid-stream batches scatter out via SWDGE (a sync-ring write
            # would head-of-line block the enc tiles queued behind it).
            # The LAST batch writes via the sync HWDGE ring instead: the
            # enc stream is done, HWDGE completes faster than SWDGE, and
            # gpsimd's teardown drain quiesces ~2us earlier.
            oeng = nc.sync if bi == bpc - 1 else nc.gpsimd
            oeng.dma_start(out=out_r[:, bi, :], in_=outsc)

    nc.finalize()
    return nc


def make_in_maps(hidden, encoder_outputs, W):
    hT_all = np.ascontiguousarray(hidden[0].T)  # [N, B]
    in_maps = []
    for c in range(NCORES):
        bsl = slice(c * BPC, (c + 1) * BPC)
        in_maps.append(
            {
                "enc": np.ascontiguousarray(encoder_outputs[:, bsl, :]),
                "hT": np.ascontiguousarray(hT_all[:, bsl]),
                "w": np.ascontiguousarray(W),
            }
        )
    return in_maps


def _install_ntff_shim():
    """The agent image's antenv package lacks axon_hooks; recreate it so
    trace=True can capture NTFF profiles. Harness runs never use this."""
    import types

    name = "antenv.axon_hooks"
    if name in sys.modules:
        return
    try:
        mod = types.ModuleType(name)
        mod._hook = None
        mod.set_axon_ntff_profile_hook = lambda h: setattr(mod, "_hook", h)
        mod.get_axon_ntff_profile_hook = lambda: mod._hook
        sys.modules[name] = mod
        if "/root/.axon_site" not in sys.path:
            sys.path.insert(0, "/root/.axon_site")
        from trn_agent_boot.trn_boot import _ntff_profile_via_ctypes

        mod._hook = _ntff_profile_via_ctypes("/opt/axon/libaxon_pjrt.so")
    except Exception:
        pass


def kernel(hidden, encoder_outputs, W, b, _trace=False, _mode="mixed"):
    """Full-input entry point. `b` (bias) is mathematically irrelevant
    (softmax shift invariance) and unused."""
    if _trace:
        _install_ntff_shim()
    from concourse.bass_utils import run_bass_kernel_spmd

    hidden = np.asarray(hidden, dtype=np.float32)
    encoder_outputs = np.asarray(encoder_outputs, dtype=np.float32)
    W = np.asarray(W, dtype=np.float32)

    nc = build(mode=_mode)
    in_maps = make_in_maps(hidden, encoder_outputs, W)
    res = run_bass_kernel_spmd(nc, in_maps, list(range(NCORES)), trace=_trace)
    full = np.concatenate([r["out"] for r in res.results], axis=0)  # [B, S]
    out = full[:, None, :].astype(np.float32)
    if _trace:
        return out, res
    return out
